# revision 118
# baseline (speedup 1.0000x reference)
"""Causal single-head attention (b=4, s=2048, d=1024, h=64) on 8 TRN2 cores.

Sharding: core c -> (batch b = c//2, g = c%2); the core owns the balanced
q-chunk pair A,B = (0,3) if g==0 else (1,2) (512 queries each).

The SPMD program is fully uniform and fully live (no masked dead work):
each core gets x regions [A, B, r0', xq, xk] where r0' is the one rest
chunk BOTH slots need, and (xq, xk) is a host-chosen duplicate pair
covering the single (slot, chunk) visit that differs per core.  Its
output accumulates into a third PSUM tile poC that the host adds into
the right slot (hosts knows g).  Visits:
    s0d: (q A,  diag A)     s1d: (q B, diag B)    s1A: (q B, k A)
    s1r: (q B,  k r0')      sC:  (q xq, k xk)

Cost-model-shaped (TimelineSim): fp8 e4m3 x/weights (x16 prescale),
DoubleRow projections (0.5 cyc/row, 256-contraction per instr); bf16
scores; causal triangles added in PSUM via fp8-DR identity x staircase
matmuls; one ACT exp per [128,2,512] score pair -> fp8 pt pair; AV as
fp8-DR over the pair (256 keys/instr) emitting [o_unnorm; denom] via a
ones-column in the padded [128,2,80] V tile.  Early rows (0:64), where
few-key softmax passes fp8 v/score errors straight through, are
recomputed exactly in a tiny bf16 patch; the host takes them from the
patch section for the chunk-0-owning cores.  PE p-state is pre-warmed
with junk matmuls so the first projections run at 2.4 GHz.
"""

import numpy as np

B, S, D, H = 4, 2048, 1024, 64
P = 128
HALF = S // 2          # 1024 queries per core
CH = 512               # free-dim chunk (PSUM bank = 512 fp32)
KT = 4                 # DoubleRow contraction tiles (256 each) over d=1024
NR = 5                 # x regions: A, B, r0', xq, xk
VW = 80                # padded AV weight cols: [v(64) | ones(1) | 0...] %16
WS = 16.0              # host weight prescale (exact power of 2)
OW = CH + (CH + H) + CH    # out cols: slot0 | slot1+patch | poC

_NC = None
TRACE = False
LAST = {}


def build_bass():
    import concourse.bass as bass  # noqa: F401
    import concourse.mybir as mybir
    import concourse.tile as tile
    from concourse import bacc
    from concourse.masks import make_identity

    f32 = mybir.dt.float32
    bf16 = mybir.dt.bfloat16
    fp8 = mybir.dt.float8e4
    AF = mybir.ActivationFunctionType
    DR = mybir.MatmulPerfMode.DoubleRow

    nc = bacc.Bacc()
    # x regions, fp8, chunk-major 4KB-contiguous: xt[p, r, k, t, s']
    xt_d = nc.dram_tensor("xt", [P, NR * 2 * KT * CH], fp8, kind="ExternalInput")
    # [wvk | wq] interleaved per DR ktile: [P, KT, 2, 192] fp8, x16-prescaled
    wall_d = nc.dram_tensor("wall", [P, 2 * KT * (P + H) + 16], fp8, kind="ExternalInput")
    msk_d = nc.dram_tensor("msk", [P, 4 * CH], fp8, kind="ExternalInput")
    # bf16 early-rows patch operands: x rows 0:64 and [Wq|Wk|Wv] unscaled
    xp_d = nc.dram_tensor("xp", [P, 8, H], bf16, kind="ExternalInput")
    wp_d = nc.dram_tensor("wp", [P, 8, 3 * H], bf16, kind="ExternalInput")
    out_d = nc.dram_tensor("out", [H + 1, 2 * CH + H], f32, kind="ExternalOutput")
    outC_d = nc.dram_tensor("outC", [H + 1, CH], bf16, kind="ExternalOutput")

    xt_r = xt_d.rearrange("p (r k t s) -> p r k t s", r=NR, k=KT, t=2)
    wall_r = wall_d[:, 0:2 * KT * (P + H)].rearrange("p (k t m) -> p k t m", k=KT, t=2)
    msk_r = msk_d.rearrange("p (j q) -> p j q", q=CH)

    with tile.TileContext(nc) as tc:
        with (
            tc.tile_pool(name="consts", bufs=1) as consts,
            tc.tile_pool(name="data", bufs=1) as data,
            tc.tile_pool(name="pp", bufs=2, space="PSUM") as pairpool,
            tc.tile_pool(name="pj", bufs=2, space="PSUM") as projpool,
            tc.tile_pool(name="po", bufs=1, space="PSUM") as popool,
            tc.tile_pool(name="pt", bufs=3) as ptpool,
        ):
            wallt = consts.tile([P, KT, 2, P + H], fp8)
            btraw = consts.tile([P, 16], fp8)
            stair = consts.tile([P, 4, CH], fp8)
            idz = consts.tile([P, 2, 2, P], fp8)   # [pair-sel][t][col]
            bt = btraw[:].bitcast(mybir.dt.float32)
            identT = consts.tile([H, H], bf16)
            xp = consts.tile([P, 8, H], bf16)
            wp = consts.tile([P, 8, 3 * H], bf16)

            xt = [data.tile([P, KT, 2, CH], fp8, tag=f"xt{r}", name=f"xt{r}")
                  for r in range(NR)]

            # DMA order tuned for stream saturation (see docstring).
            nc.sync.dma_start(xt[0][:], xt_r[:, 0])
            nc.sync.dma_start(wallt[:], wall_r[:])
            nc.sync.dma_start(btraw[:], wall_d[:, 2 * KT * (P + H):])
            nc.sync.dma_start(stair[:, 0:2, :], msk_r[:, 0:2, :])
            nc.sync.dma_start(xt[1][:], xt_r[:, 1])
            nc.sync.dma_start(xt[4][:], xt_r[:, 4])
            nc.sync.dma_start(xt[2][:], xt_r[:, 2])
            nc.sync.dma_start(xt[3][:], xt_r[:, 3])
            nc.sync.dma_start(wp[:], wp_d[:])
            nc.sync.dma_start(xp[:], xp_d[:])

            # PE p-state pre-warm on DMA-free operands.
            scratch = consts.tile([H, CH], bf16)
            make_identity(nc, identT[:])
            nc.vector.memset(scratch[:], 0.0)
            psj = projpool.tile([H, CH], f32, tag="ps", name="warm")
            for w in range(3):
                nc.tensor.matmul(psj[:], identT[:], scratch[:], start=True, stop=True)

            # idz = [I128; 0] / [0; I128] in fp8, built on the idle GpSimd
            nc.gpsimd.memset(idz[:], 0.0)
            for z in range(2):
                sl = idz[:, z, z, :]
                nc.gpsimd.memset(sl, 1.0)
                nc.gpsimd.affine_select(
                    out=sl, in_=sl,
                    compare_op=mybir.AluOpType.is_equal, fill=0.0,
                    base=0, pattern=[[-1, P]], channel_multiplier=1,
                )
            # stair j2/j3 built on the idle GpSimd (needed later than j0/j1)
            nc.gpsimd.memset(stair[:, 2:4, :], -160.0)
            for j in (2, 3):
                nc.gpsimd.affine_select(
                    out=stair[:, j, :], in_=stair[:, j, :],
                    compare_op=mybir.AluOpType.is_gt, fill=0.0,
                    base=j * P, pattern=[[-1, CH]], channel_multiplier=1,
                )

            vkt = [data.tile([P, CH], bf16, tag=f"vkt{r}", name=f"vkt{r}")
                   for r in (0, 1, 2, 4)]
            vkt = {0: vkt[0], 1: vkt[1], 2: vkt[2], 4: vkt[3]}
            vext = {r: data.tile([P, 2, 2, VW], fp8, tag=f"vx{r}", name=f"vx{r}")
                    for r in (0, 1, 2, 4)}
            qd = {r: data.tile([P, CH], bf16, tag=f"qd{r}", name=f"qd{r}")
                  for r in (0, 1, 3)}
            outsb = [data.tile([H + 1, CH], f32,
                               tag=f"outsb{s}", name=f"outsb{s}") for s in range(2)]
            outsbC = data.tile([H + 1, CH], bf16, tag="outsbC", name="outsbC")
            outsbP = data.tile([H + 1, H], f32, tag="outsbP", name="outsbP")
            qp = data.tile([H, H], bf16, tag="qp", name="qp")
            kp = data.tile([H, H], bf16, tag="kp", name="kp")
            vsp = data.tile([H, H + 1], bf16, tag="vsp", name="vsp")
            ptp = data.tile([H, H], bf16, tag="ptp", name="ptp")
            # pre-zeroed pt tiles for the trimmed diagonal pair-1 exps
            pt_trim = [data.tile([P, 2, CH], fp8, tag=f"ptt{s}", name=f"ptt{s}")
                       for s in range(2)]
            for s in range(2):
                z16 = pt_trim[s][:, :, 0:CH // 2].bitcast(mybir.dt.uint16)
                nc.vector.memset(z16, 0)

            # Primer ops: ACT exp-table load at t~0 (identT is DMA-free), and
            # early queue-semaphore observation for ACT/DVE/Pool.
            prime = consts.tile([P, 1], f32)
            nc.scalar.activation(prime[:H, :], identT[:, 0:1], AF.Exp)
            nc.vector.tensor_copy(out=prime[:], in_=bt[:, 0:1])
            nc.gpsimd.tensor_copy(out=prime[:], in_=bt[:, 1:2])
            nc.scalar.activation(prime[:], bt[:, 2:3], AF.Exp)
            nc.vector.tensor_copy(out=prime[:H], in_=stair[:H, 0, 0:1])
            nc.vector.tensor_copy(out=prime[:H], in_=stair[:H, 2, 0:1])

            # po0 and poC share a bank (tag poA): po0 is flushed long before
            # poC's first accumulation.
            po = {0: popool.tile([VW, CH], f32, tag="poA", name="po0"),
                  1: popool.tile([VW, CH], f32, tag="poB", name="po1")}
            av_n = {0: 0, 1: 0, "C": 0}
            AV_TOTAL = {0: 2, 1: 6, "C": 2}

            def proj(r, do_vk=True, q_slot=None, q_first=False):
                def qproj():
                    ps2 = projpool.tile([P, CH], f32, tag="ps", name=f"ps2_{r}")
                    for k in range(KT):
                        nc.tensor.matmul(
                            ps2[:H, :], wallt[:, k, :, P:], xt[r][:, k],
                            start=(k == 0), stop=(k == KT - 1), perf_mode=DR,
                        )
                    if r == 0:
                        # ACT evac (Identity shares the exp table): runs in
                        # ACT's pre-stream idle window, parallel to DVE.
                        nc.scalar.activation(
                            qd[r][H:, :], ps2[:H, :], AF.Identity,
                            bias=bt[H:, 1:2], scale=1.0 / WS,
                        )
                    else:
                        nc.vector.tensor_scalar(
                            out=qd[r][H:, :], in0=ps2[:H, :],
                            scalar1=1.0 / WS, scalar2=bt[H:, 1:2],
                            op0=mybir.AluOpType.mult, op1=mybir.AluOpType.add,
                        )

                if q_first and q_slot is not None:
                    qproj()
                if do_vk:
                    ps1 = projpool.tile([P, CH], f32, tag="ps", name=f"ps1_{r}")
                    for k in range(KT):
                        nc.tensor.matmul(
                            ps1[:], wallt[:, k, :, 0:P], xt[r][:, k],
                            start=(k == 0), stop=(k == KT - 1), perf_mode=DR,
                        )
                    if r == 0:
                        # split the first evac across DVE and ACT: scores
                        # pair 0 needs only key cols 0:256
                        nc.vector.tensor_scalar(
                            out=vkt[r][:, 0:CH // 2], in0=ps1[:, 0:CH // 2],
                            scalar1=1.0 / WS, scalar2=bt[:, 0:1],
                            op0=mybir.AluOpType.mult, op1=mybir.AluOpType.add,
                        )
                        nc.scalar.activation(
                            vkt[r][:, CH // 2:], ps1[:, CH // 2:], AF.Identity,
                            bias=bt[:, 0:1], scale=1.0 / WS,
                        )
                    else:
                        nc.vector.tensor_scalar(
                            out=vkt[r][:], in0=ps1[:],
                            scalar1=1.0 / WS, scalar2=bt[:, 0:1],
                            op0=mybir.AluOpType.mult, op1=mybir.AluOpType.add,
                        )
                if not q_first and q_slot is not None:
                    qproj()
                if do_vk:
                    pst = projpool.tile([P, 2, 2, H], bf16, tag="ps", name=f"pst{r}")
                    for j in range(4):
                        nc.tensor.transpose(
                            pst[:, j // 2, j % 2, :],
                            vkt[r][:H, j * P:(j + 1) * P], identT[:],
                        )
                    nc.gpsimd.memset(vext[r][:, :, :, H:VW], 0.0)
                    nc.gpsimd.memset(vext[r][:, :, :, H:H + 1], 1.0)
                    nc.vector.tensor_copy(out=vext[r][:, :, :, 0:H], in_=pst[:])

            def visit(po_key, qr, kr, diag=False):
                """One (q-region, k-region) visit: 2 score-pairs -> exps -> AV."""
                for p in range(2):
                    trim = diag and p == 1
                    lo = CH // 2 if trim else 0
                    if trim and po_key == 1:
                        # the half-width s1d trim pair borrows the poA bank,
                        # idle between flush0 and poC; this frees a pp-
                        # rotation slot for the whole back half of the stream
                        ps = popool.tile([P, 2, CH // 2], f32, tag="poA",
                                         name="ppt1")
                    else:
                        ps = pairpool.tile([P, 2, CH], f32, tag="pp",
                                           name=f"pp{po_key}_{kr}_{p}")
                    for t in range(2):
                        j = 2 * p + t
                        if diag:
                            zsel, jlo = (0, j) if j < 3 else (1, 2)
                            nc.tensor.matmul(
                                ps[:, t, 0:CH - lo], idz[:, zsel],
                                stair[:, jlo:jlo + 2, lo:CH],
                                start=True, stop=False, perf_mode=DR,
                            )
                        nc.tensor.matmul(
                            ps[:, t, 0:CH - lo],
                            vkt[kr][H:, j * P:(j + 1) * P],
                            qd[qr][H:, lo:CH],
                            start=not diag, stop=True,
                        )
                    pt = (pt_trim[po_key] if trim else
                          ptpool.tile([P, 2, CH], fp8, tag="pt",
                                      name=f"pt{po_key}_{kr}_{p}"))
                    nc.scalar.activation(
                        pt[:, :, lo:CH], ps[:, :, 0:CH - lo], AF.Exp,
                        bias=0.0, scale=0.125,
                    )
                    i = av_n[po_key]
                    nc.tensor.matmul(
                        po[po_key][:], vext[kr][:, p], pt[:],
                        start=(i == 0), stop=(i == AV_TOTAL[po_key] - 1),
                        perf_mode=DR,
                    )
                    av_n[po_key] = i + 1

            def patch():
                """bf16 recompute of rows 0:64 x keys 0:64; host uses it for
                the cores owning chunk 0."""
                psqk = projpool.tile([P, H], f32, tag="ps", name="psqk")
                for o in range(8):
                    nc.tensor.matmul(psqk[:], wp[:, o, 0:2 * H], xp[:, o, :],
                                     start=(o == 0), stop=(o == 7))
                psv = projpool.tile([H, H], f32, tag="ps", name="psv")
                for o in range(8):
                    nc.tensor.matmul(psv[:], wp[:, o, 2 * H:], xp[:, o, :],
                                     start=(o == 0), stop=(o == 7))
                nc.vector.tensor_scalar_add(qp[:], psqk[:H, :], bt[:H, 3:4])
                nc.vector.tensor_scalar_add(kp[:], psqk[H:, :], bt[H:, 3:4])
                vtp = data.tile([H, H], bf16, tag="vtp", name="vtp")
                nc.vector.tensor_scalar_add(vtp[:], psv[:], bt[:H, 0:1])
                psts = projpool.tile([H, H], bf16, tag="ps", name="psts")
                nc.tensor.transpose(psts[:], vtp[:], identT[:])
                nc.vector.memset(vsp[:, H:H + 1], 1.0)
                nc.vector.tensor_copy(out=vsp[:, 0:H], in_=psts[:])
                pss = projpool.tile([H, H], f32, tag="ps", name="pss")
                nc.tensor.matmul(pss[:], idz[:H, 0, 0, :H], stair[:H, 0, :H],
                                 start=True, stop=False)
                nc.tensor.matmul(pss[:], kp[:], qp[:], start=False, stop=True)
                nc.scalar.activation(ptp[:], pss[:], AF.Exp, bias=0.0, scale=0.125)
                pop = projpool.tile([H + 1, H], f32, tag="ps", name="pop")
                nc.tensor.matmul(pop[:], vsp[:], ptp[:], start=True, stop=True)
                nc.vector.tensor_copy(out=outsbP[:], in_=pop[:])
                nc.sync.dma_start(out_d[:, 2 * CH:2 * CH + H], outsbP[:])

            def flush(key):
                if key == "C":
                    # ACT copy: ACT is idle right after the final exp, and
                    # Copy shares the exp table.
                    nc.scalar.activation(outsbC[:], po["C"][:H + 1, :], AF.Copy)
                    nc.sync.dma_start(outC_d[:], outsbC[:])
                else:
                    nc.vector.tensor_copy(out=outsb[key][:], in_=po[key][:H + 1, :])
                    lo = 0 if key == 0 else CH
                    nc.sync.dma_start(out_d[:, lo:lo + CH], outsb[key][:])

            proj(0, q_slot=0, q_first=True)  # Q(A) then VK
            visit(0, 0, 0, diag=True)       # s0d
            proj(1, q_slot=1, q_first=True)  # Q(B) then VK
            visit(1, 1, 0)                  # s1A: q B x k A
            visit(1, 1, 1, diag=True)       # s1d
            proj(4)                         # VK(xk)
            proj(2)                         # VK only (r0')
            visit(1, 1, 2)                  # s1r: q B x k r0'
            flush(0)
            proj(3, do_vk=False, q_slot=3)  # Q(xq) -> qd[3]
            patch()
            flush(1)
            po["C"] = popool.tile([VW, CH], f32, tag="poA", name="poC")
            visit("C", 3, 4)                # sC
            flush("C")

    nc.compile()
    return nc


def make_in_maps(x, Wq, bq, Wk, bk, Wv, bv):
    import ml_dtypes
    e4 = ml_dtypes.float8_e4m3
    bf = ml_dtypes.bfloat16
    x = np.asarray(x, dtype=np.float32)

    def dr_pack(w):  # [1024, M] -> [128, KT, 2, M]
        m = w.shape[1]
        return np.ascontiguousarray(
            w.reshape(KT, 2, P, m).transpose(2, 0, 1, 3)).astype(e4)

    # [Wv|Wk|Wq] x16, interleaved per DR ktile: wall[p, k, t, :]
    wall = np.ascontiguousarray(dr_pack(np.concatenate(
        [np.asarray(Wv, np.float32), np.asarray(Wk, np.float32),
         np.asarray(Wq, np.float32)], axis=1) * WS).reshape(P, KT * 2 * (P + H)))

    # stair[p, j, q] = -160 if j*128 + p > q else 0
    pidx = np.arange(P)[:, None, None]
    jidx = np.arange(4)[None, :, None]
    qidx = np.arange(CH)[None, None, :]
    stairs = np.where(jidx * P + pidx > qidx, np.float32(-160.0), np.float32(0.0))
    msk = np.ascontiguousarray(stairs.reshape(P, 4 * CH)).astype(e4)

    # bf16 patch weights [Wq|Wk|Wv] unscaled, subtiled [128, 8, 192]
    wp = np.ascontiguousarray(
        np.concatenate([np.asarray(Wq, np.float32), np.asarray(Wk, np.float32),
                        np.asarray(Wv, np.float32)], axis=1)
        .reshape(8, P, 3 * H).transpose(1, 0, 2)).astype(bf)

    bias = np.zeros((P, 4), np.float32)
    bias[:H, 0] = np.asarray(bv, np.float32)
    bias[H:, 0] = np.asarray(bk, np.float32)
    bias[H:, 1] = np.asarray(bq, np.float32)
    bias[:, 2] = 0.0                       # exp bias (all visits live)
    bias[:H, 3] = np.asarray(bq, np.float32)
    bias[H:, 3] = np.asarray(bk, np.float32)

    in_maps = []
    for c in range(8):
        b, g = c // 2, c % 2
        A, Bc, r0, r1 = ((0, 3, 1, 2) if g == 0 else (1, 2, 0, 3))
        # s1A covers (B, A) and s1r covers (B, r0'); the per-core
        # conditional visit sC=(xq, xk) covers the remaining causal pair:
        #   g0: slot1 also needs r1        -> sC = (B, r1)
        #   g1: slot0 needs chunk 0 (=r0)  -> sC = (A, r0), s1r takes r0
        if g == 0:
            r0p, xqc, xkc = r0, Bc, r1
        else:
            r0p, xqc, xkc = r0, A, r0
        regions = (A, Bc, r0p, xqc, xkc)
        xT = x[b].T  # [1024, 2048]
        xt8 = np.stack(
            [np.ascontiguousarray(
                xT[:, cc * CH:(cc + 1) * CH].reshape(KT, 2, P, CH)
                .transpose(2, 0, 1, 3).reshape(P, 2 * KT * CH))
             for cc in regions], axis=1).reshape(P, NR * 2 * KT * CH).astype(e4)
        xp = np.ascontiguousarray(
            x[b][:H].T.reshape(8, P, H).transpose(1, 0, 2)).astype(bf)
        wall_bt = np.ascontiguousarray(np.concatenate(
            [wall, bias.astype(np.float32).view(np.uint8).view(
                np.dtype(wall.dtype))], axis=1))
        in_maps.append({"xt": np.ascontiguousarray(xt8), "wall": wall_bt,
                        "msk": msk, "xp": xp, "wp": wp})
    return in_maps


def gather(results):
    out = np.zeros((B, S, H), np.float32)
    for c in range(8):
        b, g = c // 2, c % 2
        A, Bc = (0, 3) if g == 0 else (1, 2)
        r = results[c]["out"]  # [65, 512 | 512 | 64]
        s0 = r[:, 0:CH]
        s1 = r[:, CH:CH + CH]
        pc = np.asarray(results[c]["outC"], np.float32)
        if g == 0:
            s1 = s1 + pc                   # sC covered (B, r1)
        else:
            s0 = s0 + pc                   # sC covered (A, r0)
        out[b, A * CH:(A + 1) * CH] = (s0[:H] / s0[H:H + 1]).T
        out[b, Bc * CH:(Bc + 1) * CH] = (s1[:H] / s1[H:H + 1]).T
        if A == 0:
            p = r[:, 2 * CH:2 * CH + H]  # bf16 early-rows patch
            out[b, :H] = (p[:H] / p[H:H + 1]).T
    return out


def kernel(x, Wq, bq, Wk, bk, Wv, bv):
    global _NC
    from concourse.bass_utils import run_bass_kernel_spmd

    if _NC is None:
        _NC = build_bass()
    in_maps = make_in_maps(x, Wq, bq, Wk, bk, Wv, bv)
    res = run_bass_kernel_spmd(_NC, in_maps, core_ids=list(range(8)), trace=TRACE)
    LAST["res"] = res
    return gather(res.results)


# revision 119
# speedup vs baseline: 1.0006x; 1.0006x over previous
"""Causal single-head attention (b=4, s=2048, d=1024, h=64) on 8 TRN2 cores.

Sharding: core c -> (batch b = c//2, g = c%2); the core owns the balanced
q-chunk pair A,B = (0,3) if g==0 else (1,2) (512 queries each).

The SPMD program is fully uniform and fully live (no masked dead work):
each core gets x regions [A, B, r0', xq, xk] where r0' is the one rest
chunk BOTH slots need, and (xq, xk) is a host-chosen duplicate pair
covering the single (slot, chunk) visit that differs per core.  Its
output accumulates into a third PSUM tile poC that the host adds into
the right slot (hosts knows g).  Visits:
    s0d: (q A,  diag A)     s1d: (q B, diag B)    s1A: (q B, k A)
    s1r: (q B,  k r0')      sC:  (q xq, k xk)

Cost-model-shaped (TimelineSim): fp8 e4m3 x/weights (x16 prescale),
DoubleRow projections (0.5 cyc/row, 256-contraction per instr); bf16
scores; causal triangles added in PSUM via fp8-DR identity x staircase
matmuls; one ACT exp per [128,2,512] score pair -> fp8 pt pair; AV as
fp8-DR over the pair (256 keys/instr) emitting [o_unnorm; denom] via a
ones-column in the padded [128,2,80] V tile.  Early rows (0:64), where
few-key softmax passes fp8 v/score errors straight through, are
recomputed exactly in a tiny bf16 patch; the host takes them from the
patch section for the chunk-0-owning cores.  PE p-state is pre-warmed
with junk matmuls so the first projections run at 2.4 GHz.
"""

import numpy as np

B, S, D, H = 4, 2048, 1024, 64
P = 128
HALF = S // 2          # 1024 queries per core
CH = 512               # free-dim chunk (PSUM bank = 512 fp32)
KT = 4                 # DoubleRow contraction tiles (256 each) over d=1024
NR = 5                 # x regions: A, B, r0', xq, xk
VW = 80                # padded AV weight cols: [v(64) | ones(1) | 0...] %16
WS = 16.0              # host weight prescale (exact power of 2)
OW = CH + (CH + H) + CH    # out cols: slot0 | slot1+patch | poC

_NC = None
TRACE = False
LAST = {}


def build_bass():
    import concourse.bass as bass  # noqa: F401
    import concourse.mybir as mybir
    import concourse.tile as tile
    from concourse import bacc
    from concourse.masks import make_identity

    f32 = mybir.dt.float32
    bf16 = mybir.dt.bfloat16
    fp8 = mybir.dt.float8e4
    AF = mybir.ActivationFunctionType
    DR = mybir.MatmulPerfMode.DoubleRow

    nc = bacc.Bacc()
    # x regions, fp8, chunk-major 4KB-contiguous: xt[p, r, k, t, s']
    xt_d = nc.dram_tensor("xt", [P, NR * 2 * KT * CH], fp8, kind="ExternalInput")
    # [wvk | wq] interleaved per DR ktile: [P, KT, 2, 192] fp8, x16-prescaled
    wall_d = nc.dram_tensor("wall", [P, 2 * KT * (P + H) + 16], fp8, kind="ExternalInput")
    msk_d = nc.dram_tensor("msk", [P, 4 * CH], fp8, kind="ExternalInput")
    # bf16 early-rows patch operands: x rows 0:64 and [Wq|Wk|Wv] unscaled
    xp_d = nc.dram_tensor("xp", [P, 8, H], bf16, kind="ExternalInput")
    wp_d = nc.dram_tensor("wp", [P, 8, 3 * H], bf16, kind="ExternalInput")
    out_d = nc.dram_tensor("out", [H + 1, 2 * CH + H], f32, kind="ExternalOutput")
    outC_d = nc.dram_tensor("outC", [H + 1, CH], bf16, kind="ExternalOutput")

    xt_r = xt_d.rearrange("p (r k t s) -> p r k t s", r=NR, k=KT, t=2)
    wall_r = wall_d[:, 0:2 * KT * (P + H)].rearrange("p (k t m) -> p k t m", k=KT, t=2)
    msk_r = msk_d.rearrange("p (j q) -> p j q", q=CH)

    with tile.TileContext(nc) as tc:
        with (
            tc.tile_pool(name="consts", bufs=1) as consts,
            tc.tile_pool(name="data", bufs=1) as data,
            tc.tile_pool(name="pp", bufs=2, space="PSUM") as pairpool,
            tc.tile_pool(name="pj", bufs=2, space="PSUM") as projpool,
            tc.tile_pool(name="po", bufs=1, space="PSUM") as popool,
            tc.tile_pool(name="pt", bufs=3) as ptpool,
        ):
            wallt = consts.tile([P, KT, 2, P + H], fp8)
            btraw = consts.tile([P, 16], fp8)
            stair = consts.tile([P, 4, CH], fp8)
            idz = consts.tile([P, 2, 2, P], fp8)   # [pair-sel][t][col]
            bt = btraw[:].bitcast(mybir.dt.float32)
            identT = consts.tile([H, H], bf16)
            xp = consts.tile([P, 8, H], bf16)
            wp = consts.tile([P, 8, 3 * H], bf16)

            xt = [data.tile([P, KT, 2, CH], fp8, tag=f"xt{r}", name=f"xt{r}")
                  for r in range(NR)]

            # DMA order tuned for stream saturation (see docstring).
            nc.sync.dma_start(xt[0][:], xt_r[:, 0])
            nc.sync.dma_start(wallt[:], wall_r[:])
            nc.sync.dma_start(btraw[:], wall_d[:, 2 * KT * (P + H):])
            nc.sync.dma_start(stair[:, 0:2, :], msk_r[:, 0:2, :])
            nc.sync.dma_start(xt[1][:], xt_r[:, 1])
            nc.sync.dma_start(xt[4][:], xt_r[:, 4])
            nc.sync.dma_start(xt[2][:], xt_r[:, 2])
            nc.sync.dma_start(xt[3][:], xt_r[:, 3])
            nc.sync.dma_start(wp[:], wp_d[:])
            nc.sync.dma_start(xp[:], xp_d[:])

            # PE p-state pre-warm on DMA-free operands.
            scratch = consts.tile([H, CH], bf16)
            make_identity(nc, identT[:])
            nc.vector.memset(scratch[:], 0.0)
            psj = projpool.tile([H, CH], f32, tag="ps", name="warm")
            for w in range(3):
                nc.tensor.matmul(psj[:], identT[:], scratch[:], start=True, stop=True)

            # idz = [I128; 0] / [0; I128] in fp8, built on the idle GpSimd
            nc.gpsimd.memset(idz[:], 0.0)
            for z in range(2):
                sl = idz[:, z, z, :]
                nc.gpsimd.memset(sl, 1.0)
                nc.gpsimd.affine_select(
                    out=sl, in_=sl,
                    compare_op=mybir.AluOpType.is_equal, fill=0.0,
                    base=0, pattern=[[-1, P]], channel_multiplier=1,
                )
            # stair j2/j3 built on the idle GpSimd (needed later than j0/j1)
            nc.gpsimd.memset(stair[:, 2:4, :], -160.0)
            for j in (2, 3):
                nc.gpsimd.affine_select(
                    out=stair[:, j, :], in_=stair[:, j, :],
                    compare_op=mybir.AluOpType.is_gt, fill=0.0,
                    base=j * P, pattern=[[-1, CH]], channel_multiplier=1,
                )

            vkt = [data.tile([P, CH], bf16, tag=f"vkt{r}", name=f"vkt{r}")
                   for r in (0, 1, 2, 4)]
            vkt = {0: vkt[0], 1: vkt[1], 2: vkt[2], 4: vkt[3]}
            vext = {r: data.tile([P, 2, 2, VW], fp8, tag=f"vx{r}", name=f"vx{r}")
                    for r in (0, 1, 2, 4)}
            qd = {r: data.tile([P, CH], bf16, tag=f"qd{r}", name=f"qd{r}")
                  for r in (0, 1, 3)}
            outsb = [data.tile([H + 1, CH], f32,
                               tag=f"outsb{s}", name=f"outsb{s}") for s in range(2)]
            outsbC = data.tile([H + 1, CH], bf16, tag="outsbC", name="outsbC")
            outsbP = data.tile([H + 1, H], f32, tag="outsbP", name="outsbP")
            qp = data.tile([H, H], bf16, tag="qp", name="qp")
            kp = data.tile([H, H], bf16, tag="kp", name="kp")
            vsp = data.tile([H, H + 1], bf16, tag="vsp", name="vsp")
            ptp = data.tile([H, H], bf16, tag="ptp", name="ptp")
            # pre-zeroed pt tiles for the trimmed diagonal pair-1 exps
            pt_trim = [data.tile([P, 2, CH], fp8, tag=f"ptt{s}", name=f"ptt{s}")
                       for s in range(2)]
            for s in range(2):
                z16 = pt_trim[s][:, :, 0:CH // 2].bitcast(mybir.dt.uint16)
                nc.vector.memset(z16, 0)

            # Primer ops: ACT exp-table load at t~0 (identT is DMA-free), and
            # early queue-semaphore observation for ACT/DVE/Pool.
            prime = consts.tile([P, 1], f32)
            nc.scalar.activation(prime[:H, :], identT[:, 0:1], AF.Exp)
            nc.vector.tensor_copy(out=prime[:], in_=bt[:, 0:1])
            nc.gpsimd.tensor_copy(out=prime[:], in_=bt[:, 1:2])
            nc.scalar.activation(prime[:], bt[:, 2:3], AF.Exp)
            nc.vector.tensor_copy(out=prime[:H], in_=stair[:H, 0, 0:1])
            nc.vector.tensor_copy(out=prime[:H], in_=stair[:H, 2, 0:1])

            # po0 and poC share a bank (tag poA): po0 is flushed long before
            # poC's first accumulation.
            po = {0: popool.tile([VW, CH], f32, tag="poA", name="po0"),
                  1: popool.tile([VW, CH], f32, tag="poB", name="po1")}
            av_n = {0: 0, 1: 0, "C": 0}
            AV_TOTAL = {0: 2, 1: 6, "C": 2}

            def proj(r, do_vk=True, q_slot=None, q_first=False):
                def qproj():
                    ps2 = projpool.tile([P, CH], f32, tag="ps", name=f"ps2_{r}")
                    for k in range(KT):
                        nc.tensor.matmul(
                            ps2[:H, :], wallt[:, k, :, P:], xt[r][:, k],
                            start=(k == 0), stop=(k == KT - 1), perf_mode=DR,
                        )
                    if r == 0:
                        # ACT evac (Identity shares the exp table): runs in
                        # ACT's pre-stream idle window, parallel to DVE.
                        nc.scalar.activation(
                            qd[r][H:, :], ps2[:H, :], AF.Identity,
                            bias=bt[H:, 1:2], scale=1.0 / WS,
                        )
                    else:
                        nc.vector.tensor_scalar(
                            out=qd[r][H:, :], in0=ps2[:H, :],
                            scalar1=1.0 / WS, scalar2=bt[H:, 1:2],
                            op0=mybir.AluOpType.mult, op1=mybir.AluOpType.add,
                        )

                if q_first and q_slot is not None:
                    qproj()
                if do_vk:
                    ps1 = projpool.tile([P, CH], f32, tag="ps", name=f"ps1_{r}")
                    for k in range(KT):
                        nc.tensor.matmul(
                            ps1[:], wallt[:, k, :, 0:P], xt[r][:, k],
                            start=(k == 0), stop=(k == KT - 1), perf_mode=DR,
                        )
                    if r == 0:
                        # split the first evac across DVE and ACT: scores
                        # pair 0 needs only key cols 0:256
                        nc.vector.tensor_scalar(
                            out=vkt[r][:, 0:CH // 2], in0=ps1[:, 0:CH // 2],
                            scalar1=1.0 / WS, scalar2=bt[:, 0:1],
                            op0=mybir.AluOpType.mult, op1=mybir.AluOpType.add,
                        )
                        nc.scalar.activation(
                            vkt[r][:, CH // 2:], ps1[:, CH // 2:], AF.Identity,
                            bias=bt[:, 0:1], scale=1.0 / WS,
                        )
                    else:
                        nc.vector.tensor_scalar(
                            out=vkt[r][:], in0=ps1[:],
                            scalar1=1.0 / WS, scalar2=bt[:, 0:1],
                            op0=mybir.AluOpType.mult, op1=mybir.AluOpType.add,
                        )
                if not q_first and q_slot is not None:
                    qproj()
                if do_vk:
                    pst = projpool.tile([P, 2, 2, H], bf16, tag="ps", name=f"pst{r}")
                    for j in range(4):
                        nc.tensor.transpose(
                            pst[:, j // 2, j % 2, :],
                            vkt[r][:H, j * P:(j + 1) * P], identT[:],
                        )
                    nc.gpsimd.memset(vext[r][:, :, :, H:VW], 0.0)
                    nc.gpsimd.memset(vext[r][:, :, :, H:H + 1], 1.0)
                    nc.vector.tensor_copy(out=vext[r][:, :, :, 0:H], in_=pst[:])

            def visit(po_key, qr, kr, diag=False):
                """One (q-region, k-region) visit: 2 score-pairs -> exps -> AV."""
                for p in range(2):
                    trim = diag and p == 1
                    lo = CH // 2 if trim else 0
                    if trim and po_key == 1:
                        # the half-width s1d trim pair borrows the poA bank,
                        # idle between flush0 and poC; this frees a pp-
                        # rotation slot for the whole back half of the stream
                        ps = popool.tile([P, 2, CH // 2], f32, tag="poA",
                                         name="ppt1")
                    else:
                        ps = pairpool.tile([P, 2, CH], f32, tag="pp",
                                           name=f"pp{po_key}_{kr}_{p}")
                    for t in range(2):
                        j = 2 * p + t
                        if diag:
                            zsel, jlo = (0, j) if j < 3 else (1, 2)
                            nc.tensor.matmul(
                                ps[:, t, 0:CH - lo], idz[:, zsel],
                                stair[:, jlo:jlo + 2, lo:CH],
                                start=True, stop=False, perf_mode=DR,
                            )
                        nc.tensor.matmul(
                            ps[:, t, 0:CH - lo],
                            vkt[kr][H:, j * P:(j + 1) * P],
                            qd[qr][H:, lo:CH],
                            start=not diag, stop=True,
                        )
                    pt = (pt_trim[po_key] if trim else
                          ptpool.tile([P, 2, CH], fp8, tag="pt",
                                      name=f"pt{po_key}_{kr}_{p}"))
                    nc.scalar.activation(
                        pt[:, :, lo:CH], ps[:, :, 0:CH - lo], AF.Exp,
                        bias=0.0, scale=0.125,
                    )
                    i = av_n[po_key]
                    nc.tensor.matmul(
                        po[po_key][:], vext[kr][:, p], pt[:],
                        start=(i == 0), stop=(i == AV_TOTAL[po_key] - 1),
                        perf_mode=DR,
                    )
                    av_n[po_key] = i + 1

            def patch():
                """bf16 recompute of rows 0:64 x keys 0:64; host uses it for
                the cores owning chunk 0."""
                psqk = projpool.tile([P, H], f32, tag="ps", name="psqk")
                for o in range(8):
                    nc.tensor.matmul(psqk[:], wp[:, o, 0:2 * H], xp[:, o, :],
                                     start=(o == 0), stop=(o == 7))
                psv = projpool.tile([H, H], f32, tag="ps", name="psv")
                for o in range(8):
                    nc.tensor.matmul(psv[:], wp[:, o, 2 * H:], xp[:, o, :],
                                     start=(o == 0), stop=(o == 7))
                nc.vector.tensor_scalar_add(qp[:], psqk[:H, :], bt[:H, 3:4])
                nc.vector.tensor_scalar_add(kp[:], psqk[H:, :], bt[H:, 3:4])
                vtp = data.tile([H, H], bf16, tag="vtp", name="vtp")
                nc.vector.tensor_scalar_add(vtp[:], psv[:], bt[:H, 0:1])
                psts = projpool.tile([H, H], bf16, tag="ps", name="psts")
                nc.tensor.transpose(psts[:], vtp[:], identT[:])
                nc.vector.memset(vsp[:, H:H + 1], 1.0)
                nc.vector.tensor_copy(out=vsp[:, 0:H], in_=psts[:])
                pss = projpool.tile([H, H], f32, tag="ps", name="pss")
                nc.tensor.matmul(pss[:], idz[:H, 0, 0, :H], stair[:H, 0, :H],
                                 start=True, stop=False)
                nc.tensor.matmul(pss[:], kp[:], qp[:], start=False, stop=True)
                nc.scalar.activation(ptp[:], pss[:], AF.Exp, bias=0.0, scale=0.125)
                pop = projpool.tile([H + 1, H], f32, tag="ps", name="pop")
                nc.tensor.matmul(pop[:], vsp[:], ptp[:], start=True, stop=True)
                nc.vector.tensor_copy(out=outsbP[:], in_=pop[:])
                nc.sync.dma_start(out_d[:, 2 * CH:2 * CH + H], outsbP[:])

            def flush(key):
                if key == "C":
                    # ACT copy: ACT is idle right after the final exp, and
                    # Copy shares the exp table.
                    nc.vector.tensor_copy(out=outsbC[:], in_=po["C"][:H + 1, :])
                    nc.sync.dma_start(outC_d[:], outsbC[:])
                else:
                    nc.vector.tensor_copy(out=outsb[key][:], in_=po[key][:H + 1, :])
                    lo = 0 if key == 0 else CH
                    nc.sync.dma_start(out_d[:, lo:lo + CH], outsb[key][:])

            proj(0, q_slot=0, q_first=True)  # Q(A) then VK
            visit(0, 0, 0, diag=True)       # s0d
            proj(1, q_slot=1, q_first=True)  # Q(B) then VK
            visit(1, 1, 0)                  # s1A: q B x k A
            visit(1, 1, 1, diag=True)       # s1d
            proj(4)                         # VK(xk)
            proj(2)                         # VK only (r0')
            visit(1, 1, 2)                  # s1r: q B x k r0'
            flush(0)
            proj(3, do_vk=False, q_slot=3)  # Q(xq) -> qd[3]
            patch()
            flush(1)
            po["C"] = popool.tile([VW, CH], f32, tag="poA", name="poC")
            visit("C", 3, 4)                # sC
            flush("C")

    nc.compile()
    return nc


def make_in_maps(x, Wq, bq, Wk, bk, Wv, bv):
    import ml_dtypes
    e4 = ml_dtypes.float8_e4m3
    bf = ml_dtypes.bfloat16
    x = np.asarray(x, dtype=np.float32)

    def dr_pack(w):  # [1024, M] -> [128, KT, 2, M]
        m = w.shape[1]
        return np.ascontiguousarray(
            w.reshape(KT, 2, P, m).transpose(2, 0, 1, 3)).astype(e4)

    # [Wv|Wk|Wq] x16, interleaved per DR ktile: wall[p, k, t, :]
    wall = np.ascontiguousarray(dr_pack(np.concatenate(
        [np.asarray(Wv, np.float32), np.asarray(Wk, np.float32),
         np.asarray(Wq, np.float32)], axis=1) * WS).reshape(P, KT * 2 * (P + H)))

    # stair[p, j, q] = -160 if j*128 + p > q else 0
    pidx = np.arange(P)[:, None, None]
    jidx = np.arange(4)[None, :, None]
    qidx = np.arange(CH)[None, None, :]
    stairs = np.where(jidx * P + pidx > qidx, np.float32(-160.0), np.float32(0.0))
    msk = np.ascontiguousarray(stairs.reshape(P, 4 * CH)).astype(e4)

    # bf16 patch weights [Wq|Wk|Wv] unscaled, subtiled [128, 8, 192]
    wp = np.ascontiguousarray(
        np.concatenate([np.asarray(Wq, np.float32), np.asarray(Wk, np.float32),
                        np.asarray(Wv, np.float32)], axis=1)
        .reshape(8, P, 3 * H).transpose(1, 0, 2)).astype(bf)

    bias = np.zeros((P, 4), np.float32)
    bias[:H, 0] = np.asarray(bv, np.float32)
    bias[H:, 0] = np.asarray(bk, np.float32)
    bias[H:, 1] = np.asarray(bq, np.float32)
    bias[:, 2] = 0.0                       # exp bias (all visits live)
    bias[:H, 3] = np.asarray(bq, np.float32)
    bias[H:, 3] = np.asarray(bk, np.float32)

    in_maps = []
    for c in range(8):
        b, g = c // 2, c % 2
        A, Bc, r0, r1 = ((0, 3, 1, 2) if g == 0 else (1, 2, 0, 3))
        # s1A covers (B, A) and s1r covers (B, r0'); the per-core
        # conditional visit sC=(xq, xk) covers the remaining causal pair:
        #   g0: slot1 also needs r1        -> sC = (B, r1)
        #   g1: slot0 needs chunk 0 (=r0)  -> sC = (A, r0), s1r takes r0
        if g == 0:
            r0p, xqc, xkc = r0, Bc, r1
        else:
            r0p, xqc, xkc = r0, A, r0
        regions = (A, Bc, r0p, xqc, xkc)
        xT = x[b].T  # [1024, 2048]
        xt8 = np.stack(
            [np.ascontiguousarray(
                xT[:, cc * CH:(cc + 1) * CH].reshape(KT, 2, P, CH)
                .transpose(2, 0, 1, 3).reshape(P, 2 * KT * CH))
             for cc in regions], axis=1).reshape(P, NR * 2 * KT * CH).astype(e4)
        xp = np.ascontiguousarray(
            x[b][:H].T.reshape(8, P, H).transpose(1, 0, 2)).astype(bf)
        wall_bt = np.ascontiguousarray(np.concatenate(
            [wall, bias.astype(np.float32).view(np.uint8).view(
                np.dtype(wall.dtype))], axis=1))
        in_maps.append({"xt": np.ascontiguousarray(xt8), "wall": wall_bt,
                        "msk": msk, "xp": xp, "wp": wp})
    return in_maps


def gather(results):
    out = np.zeros((B, S, H), np.float32)
    for c in range(8):
        b, g = c // 2, c % 2
        A, Bc = (0, 3) if g == 0 else (1, 2)
        r = results[c]["out"]  # [65, 512 | 512 | 64]
        s0 = r[:, 0:CH]
        s1 = r[:, CH:CH + CH]
        pc = np.asarray(results[c]["outC"], np.float32)
        if g == 0:
            s1 = s1 + pc                   # sC covered (B, r1)
        else:
            s0 = s0 + pc                   # sC covered (A, r0)
        out[b, A * CH:(A + 1) * CH] = (s0[:H] / s0[H:H + 1]).T
        out[b, Bc * CH:(Bc + 1) * CH] = (s1[:H] / s1[H:H + 1]).T
        if A == 0:
            p = r[:, 2 * CH:2 * CH + H]  # bf16 early-rows patch
            out[b, :H] = (p[:H] / p[H:H + 1]).T
    return out


def kernel(x, Wq, bq, Wk, bk, Wv, bv):
    global _NC
    from concourse.bass_utils import run_bass_kernel_spmd

    if _NC is None:
        _NC = build_bass()
    in_maps = make_in_maps(x, Wq, bq, Wk, bk, Wv, bv)
    res = run_bass_kernel_spmd(_NC, in_maps, core_ids=list(range(8)), trace=TRACE)
    LAST["res"] = res
    return gather(res.results)
